# revision 19
# baseline (speedup 1.0000x reference)
"""Multi-head attention kernel for Trainium2 (Bass/Tile), 8-core data parallel.

Problem: B=32, N=1024, D=512, H=8 (per-head dim = D = 512).
Reference: kh = k@Wk.T+bk; qh = q@Wq.T+bq; vh = v@Wv.T+bv
  S = qh@kh.T/sqrt(D); P = softmax(S); out_h = P@vh
  rep = concat_interleaved(out_h) @ Wo.T + bo

Algebraic fusion (host-side, fp64) removes the K and V projections:
  S_ij ~ k_j . (A_h q_i + u_h)   with A_h = Wk_h^T Wq_h, u_h = Wk_h^T bq_h
         (i-only and constant terms dropped: softmax-invariant)
  rep  = sum_h G_h (P_h v) + bo_eff   with G_h = Wo_h Wv_h,
         bo_eff = bo + sum_h Wo_h bv_h   (P_h rows sum to 1)
so the device only computes, per (b, h):
  qMT[e,i] = matmul(lhsT=AT_h, rhs=qT)        (+ u during PSUM eviction)
  ST[j,i]  = matmul(lhsT=kT,  rhs=qMT)        (raw k is the stationary side)
  E[j,i]   = exp(ST/sqrt(D))                  (no max-subtract: scores ~N(0,1))
  denom    = onesT @ E   (all-ones lhsT -> every row = column sum)
  outT[d,i]= matmul(lhsT=vN,  rhs=E) * (1/denom)  (raw v natural layout)
  repT[e,i]+= matmul(lhsT=GT_h, rhs=outT)     (accumulate heads in SBUF)
  out = repT + bo_eff

All matmul operands are bfloat16 (fp32 PSUM accumulate): same PE issue rate
as fp32r but ~half the power/SBUF/DMA; rel err ~6e-3 vs 2e-2 budget.
Sharding: batch data-parallel, 4 batches per core. A/G weight stacks (8 MB
bf16 total) stay SBUF-resident across the batch loop.
"""
import math
from contextlib import ExitStack

import ml_dtypes
import numpy as np

import concourse.bacc as bacc
import concourse.mybir as mybir
import concourse.tile as tile
from concourse.bass_utils import run_bass_kernel_spmd

dt = mybir.dt
P = 128

B, N, D, H = 32, 1024, 512, 8
NCORES = 8
BLOC = B // NCORES

FD = 512           # matmul free-dim / PSUM bank width (f32)
SCALE = 1.0 / math.sqrt(D)

MMDT = "bfloat16"


class _Ctx:
    pass


def build_core_program(bloc=BLOC, n=N, d=D, h_cnt=H, reps=1, pe_only=False,
                       ps_s_bufs=3, ps_pv_bufs=4, mmdt=MMDT, den_mode="tree"):
    """Bass program for one core: bloc batches, full heads."""
    c = _Ctx()
    c.DC = d // P        # d-partition chunks (4)
    c.EC = d // P        # output-feature chunks (4)
    c.IC = n // FD       # query free-dim chunks (2)
    c.JC8 = n // P       # key partition chunks (8)
    c.n, c.d, c.h_cnt = n, d, h_cnt
    c.pe_only = pe_only
    c.den_mode = den_mode

    nc = bacc.Bacc("TRN2", target_bir_lowering=False, debug=False)
    c.nc = nc

    f32 = dt.float32
    mdt = getattr(dt, mmdt)
    c.f32, c.mdt = f32, mdt
    c.qT = nc.dram_tensor("qT", [bloc, d, n], mdt, kind="ExternalInput")
    c.kT = nc.dram_tensor("kT", [bloc, d, n], mdt, kind="ExternalInput")
    c.vN = nc.dram_tensor("vN", [bloc, n, d], mdt, kind="ExternalInput")
    c.AT = nc.dram_tensor("AT", [h_cnt, d, d], mdt, kind="ExternalInput")
    c.GT = nc.dram_tensor("GT", [h_cnt, d, d], mdt, kind="ExternalInput")
    c.u_d = nc.dram_tensor("u_d", [P, h_cnt * c.EC], f32, kind="ExternalInput")
    c.bo_d = nc.dram_tensor("bo_d", [P, c.EC], f32, kind="ExternalInput")
    c.ones_d = nc.dram_tensor("ones_d", [P, P], mdt, kind="ExternalInput")
    c.outT = nc.dram_tensor("outT", [bloc, d, n], f32, kind="ExternalOutput")

    c.AF = mybir.ActivationFunctionType

    with tile.TileContext(nc) as tc, ExitStack() as es:
        ep = es.enter_context
        c.const = ep(tc.tile_pool(name="const", bufs=1))
        c.acts = ep(tc.tile_pool(name="acts", bufs=1))
        c.projp = ep(tc.tile_pool(name="proj", bufs=2))
        c.esbp = ep(tc.tile_pool(name="esb", bufs=3))
        c.treep = ep(tc.tile_pool(name="tree", bufs=2))
        c.outnp = ep(tc.tile_pool(name="outn", bufs=2))
        c.recipp = ep(tc.tile_pool(name="recip", bufs=2))
        c.repp = ep(tc.tile_pool(name="rep", bufs=1))
        c.ps_s = ep(tc.tile_pool(name="ps_s", bufs=ps_s_bufs, space="PSUM"))
        c.ps_pv = ep(tc.tile_pool(name="ps_pv", bufs=ps_pv_bufs, space="PSUM"))
        c.ps_d = ep(tc.tile_pool(name="ps_d", bufs=1, space="PSUM"))

        c.ones = c.const.tile([P, P], mdt, name="ones")
        nc.sync.dma_start(c.ones[:], c.ones_d[:])
        c.u_sb = c.const.tile([P, h_cnt * c.EC], f32, name="u_sb")
        nc.sync.dma_start(c.u_sb[:], c.u_d[:])
        c.bo_sb = c.const.tile([P, c.EC], f32, name="bo_sb")
        nc.sync.dma_start(c.bo_sb[:], c.bo_d[:])

        # resident weight stacks: AT (qM projection) + GT (output projection)
        c.wa = c.const.tile([P, h_cnt, c.DC, d], mdt, name="wa")
        c.wg = c.const.tile([P, h_cnt, c.DC, d], mdt, name="wg")

        if pe_only:
            c.d_qMT = c.const.tile([P, c.EC, n], mdt, name="d_qMT")
            c.d_e = c.const.tile([P, c.JC8, FD], mdt, name="d_e")
            c.d_outn = c.const.tile([P, c.DC, FD], mdt, name="d_outn")
            nc.sync.dma_start(c.d_qMT[:], c.qT[0].rearrange("(c p) n -> p c n", p=P))
            for jcx in range(c.JC8):
                nc.sync.dma_start(c.d_e[:, jcx, :], c.qT[0, 0:P, 0:FD])
            for dcx in range(c.DC):
                nc.sync.dma_start(c.d_outn[:, dcx, :], c.qT[0, 0:P, 0:FD])

        for rep in range(reps):
            for b in range(bloc):
                _emit_batch(c, b, first=(rep == 0 and b == 0))

    nc.compile()
    return nc


def _emit_batch(c, b, first=False):
    nc = c.nc
    qt = c.acts.tile([P, c.DC, c.n], c.mdt, name="qt")
    kt = c.acts.tile([P, c.DC, c.n], c.mdt, name="kt")
    vn = c.acts.tile([P, c.JC8, c.d], c.mdt, name="vn")
    # HWDGE queue is serial: issue in first-use order. On the first batch,
    # interleave the one-time weight loads so head 0's matrices come first.
    if first:
        nc.sync.dma_start(c.wa[:, 0], c.AT[0].rearrange("(c p) e -> p c e", p=P))
    for dcx in range(c.DC):
        nc.sync.dma_start(qt[:, dcx, :], c.qT[b, dcx * P:(dcx + 1) * P, :])
        nc.sync.dma_start(kt[:, dcx, :], c.kT[b, dcx * P:(dcx + 1) * P, :])
    for jc8 in range(c.JC8):
        nc.sync.dma_start(vn[:, jc8, :], c.vN[b, jc8 * P:(jc8 + 1) * P, :])
    if first:
        nc.sync.dma_start(c.wg[:, 0], c.GT[0].rearrange("(c p) e -> p c e", p=P))
        for h in range(1, c.h_cnt):
            nc.sync.dma_start(c.wa[:, h], c.AT[h].rearrange("(c p) e -> p c e", p=P))
            nc.sync.dma_start(c.wg[:, h], c.GT[h].rearrange("(c p) e -> p c e", p=P))

    repT = c.repp.tile([P, c.EC, c.n], c.f32, name="repT")

    # software-pipelined head loop: the qM projection for head h+1 is
    # emitted between head h's S phase and its PV/G tail, so the PE has
    # work while the DVE tree / PSUM evictions for head h complete.
    qMT = _emit_proj(c, 0, qt)
    for h in range(c.h_cnt):
        sctx = [_emit_s_phase(c, h, ic, qMT, kt) for ic in range(c.IC)]
        qMT_next = _emit_proj(c, h + 1, qt) if h + 1 < c.h_cnt else None
        for ic in range(c.IC):
            _emit_tail(c, h, ic, sctx[ic], vn, repT)
        qMT = qMT_next

    for ec in range(c.EC):
        nc.vector.tensor_scalar_add(
            repT[:, ec, :], repT[:, ec, :], c.bo_sb[:, ec:ec + 1])
        nc.sync.dma_start(
            c.outT[b, ec * P:(ec + 1) * P, :], repT[:, ec, :])


def _emit_proj(c, h, qt):
    """qM projection: qMT[e, i] = AT_h^T @ qT (+ u bias on eviction)."""
    nc = c.nc
    DC, EC, IC = c.DC, c.EC, c.IC
    if c.pe_only:
        qMT = c.d_qMT
    else:
        qMT = c.projp.tile([P, EC, c.n], c.mdt, name="qMT")
    for ec in range(EC):
        for ic in range(IC):
            pq = c.ps_s.tile([P, FD], c.f32, name="ps_s")
            for dc in range(DC):
                nc.tensor.matmul(
                    pq[:], c.wa[:, h, dc, ec * P:(ec + 1) * P],
                    qt[:, dc, ic * FD:(ic + 1) * FD],
                    start=(dc == 0), stop=(dc == DC - 1))
            if not c.pe_only:
                nc.scalar.activation(
                    qMT[:, ec, ic * FD:(ic + 1) * FD], pq[:], c.AF.Identity,
                    bias=c.u_sb[:, h * EC + ec:h * EC + ec + 1])
    return qMT


def _emit_s_phase(c, h, ic, qMT, kt):
    """S(j) on PE, exp(j) on ACT pipelined into one contiguous E tile,
    denominator partial sums on DVE (pairwise, so they trail the exps)."""
    nc = c.nc
    EC, JC8 = c.EC, c.JC8
    i_sl = slice(ic * FD, (ic + 1) * FD)

    e_big = c.d_e if c.pe_only else c.esbp.tile([P, JC8, FD], c.mdt, name="e_big")
    for jc8 in range(JC8):
        st = c.ps_s.tile([P, FD], c.f32, name="ps_s")
        for ec in range(EC):
            nc.tensor.matmul(
                st[:], kt[:, ec, jc8 * P:(jc8 + 1) * P], qMT[:, ec, i_sl],
                start=(ec == 0), stop=(ec == EC - 1))
        if not c.pe_only:
            nc.scalar.activation(e_big[:, jc8, :], st[:], c.AF.Exp, scale=SCALE)

    t4 = None
    if not c.pe_only and c.den_mode == "tree":
        t4 = c.treep.tile([P, 7, FD], c.mdt, name="t4")
        nc.vector.tensor_add(t4[:, 0, :], e_big[:, 0, :], e_big[:, 1, :])
        nc.vector.tensor_add(t4[:, 1, :], e_big[:, 2, :], e_big[:, 3, :])
        nc.vector.tensor_add(t4[:, 4, :], t4[:, 0, :], t4[:, 1, :])
        nc.vector.tensor_add(t4[:, 2, :], e_big[:, 4, :], e_big[:, 5, :])
        nc.vector.tensor_add(t4[:, 3, :], e_big[:, 6, :], e_big[:, 7, :])
        nc.vector.tensor_add(t4[:, 5, :], t4[:, 2, :], t4[:, 3, :])
        nc.vector.tensor_add(t4[:, 6, :], t4[:, 4, :], t4[:, 5, :])
    return e_big, t4


def _emit_tail(c, h, ic, sctx, vn, repT):
    """den matmul + PV + normalization + output projection for (h, ic)."""
    nc = c.nc
    DC, EC, JC8 = c.DC, c.EC, c.JC8
    i_sl = slice(ic * FD, (ic + 1) * FD)
    e_big, t4 = sctx

    pv_ps = [c.ps_pv.tile([P, FD], c.f32, name="ps_pv") for _ in range(DC)]
    den_ps = c.ps_d.tile([P, FD], c.f32, name="ps_d")

    if not c.pe_only:
        if c.den_mode == "tree":
            nc.tensor.matmul(den_ps[:], c.ones[:], t4[:, 6, :],
                             start=True, stop=True)
        else:
            for jc8 in range(JC8):
                nc.tensor.matmul(den_ps[:], c.ones[:], e_big[:, jc8, :],
                                 start=(jc8 == 0), stop=(jc8 == JC8 - 1))

    if c.pe_only:
        outn = c.d_outn
        for dc in range(DC):
            for jc8 in range(JC8):
                nc.tensor.matmul(
                    pv_ps[dc][:], vn[:, jc8, dc * P:(dc + 1) * P],
                    e_big[:, jc8, :],
                    start=(jc8 == 0), stop=(jc8 == JC8 - 1))
    else:
        recip = c.recipp.tile([P, FD], c.f32, name="recip")
        nc.vector.reciprocal(recip[:], den_ps[:])
        outn = c.outnp.tile([P, DC, FD], c.mdt, name="outn")
        for dc in range(DC):
            for jc8 in range(JC8):
                nc.tensor.matmul(
                    pv_ps[dc][:], vn[:, jc8, dc * P:(dc + 1) * P],
                    e_big[:, jc8, :],
                    start=(jc8 == 0), stop=(jc8 == JC8 - 1))
            nc.vector.tensor_mul(outn[:, dc, :], pv_ps[dc][:], recip[:])

    # output projection for this (h, ic)
    for ec in range(EC):
        po = c.ps_d.tile([P, FD], c.f32, name="ps_d")
        for dc in range(DC):
            nc.tensor.matmul(
                po[:], c.wg[:, h, dc, ec * P:(ec + 1) * P], outn[:, dc, :],
                start=(dc == 0), stop=(dc == DC - 1))
        if c.pe_only:
            continue
        if h == 0:
            nc.vector.tensor_copy(repT[:, ec, i_sl], po[:])
        else:
            nc.vector.tensor_add(repT[:, ec, i_sl], repT[:, ec, i_sl], po[:])


_CACHED_NC = None


def _get_nc():
    global _CACHED_NC
    if _CACHED_NC is None:
        _CACHED_NC = build_core_program()
    return _CACHED_NC


def _prep_in_maps(q, k, v, Wq, bq, Wk, bk, Wv, bv, Wo, bo):
    """Host-side layout prep, weight fusion (fp64) + sharding."""
    f32, f64 = np.float32, np.float64
    bf16 = ml_dtypes.bfloat16
    qT = np.ascontiguousarray(
        q.reshape(NCORES, BLOC, N, D).transpose(0, 1, 3, 2)).astype(bf16)
    kT = np.ascontiguousarray(
        k.reshape(NCORES, BLOC, N, D).transpose(0, 1, 3, 2)).astype(bf16)
    vN = np.ascontiguousarray(v.reshape(NCORES, BLOC, N, D)).astype(bf16)

    Wq64, Wk64 = Wq.astype(f64), Wk.astype(f64)
    Wv64, bv64 = Wv.astype(f64), bv.astype(f64)
    # AT[h] = (A_h)^T with A_h = Wk_h^T Wq_h  -> lhsT layout [d_in, e_out]
    AT = np.einsum("hea,heb->hab", Wq64, Wk64)       # = Wq^T Wk = A^T
    # WoT[h, dd, eo] = Wo[eo, dd*H + h]
    WoT64 = Wo.astype(f64).reshape(D, D, H).transpose(2, 1, 0)
    # GT[h] = (G_h)^T = Wv_h^T WoT64[h]  -> lhsT layout [d_in, e_out]
    GT = np.einsum("hda,hde->hae", Wv64, WoT64)
    # u_h = Wk_h^T bq_h ; device layout u_dev[p, h*EC+ec] = u[h, ec*128+p]
    u = np.einsum("hed,he->hd", Wk64, bq.astype(f64))
    u_dev = np.ascontiguousarray(
        u.reshape(H, D // P, P).transpose(2, 0, 1).reshape(P, -1)).astype(f32)
    # bo_eff = bo + sum_h Wo_h bv_h   (softmax rows sum to 1)
    bo_eff = bo.astype(f64) + np.einsum("hd,hde->e", bv64, WoT64)
    bo_dev = np.ascontiguousarray(
        bo_eff.reshape(D // P, P).T).astype(f32)
    ones = np.ones((P, P), bf16)

    shared = dict(AT=AT.astype(bf16), GT=GT.astype(bf16),
                  u_d=u_dev, bo_d=bo_dev, ones_d=ones)
    return [dict(qT=qT[c], kT=kT[c], vN=vN[c], **shared) for c in range(NCORES)]


def kernel(**inputs):
    nc = _get_nc()
    in_maps = _prep_in_maps(
        inputs["q"], inputs["k"], inputs["v"],
        inputs["Wq"], inputs["bq"], inputs["Wk"], inputs["bk"],
        inputs["Wv"], inputs["bv"], inputs["Wo"], inputs["bo"])
    res = run_bass_kernel_spmd(nc, in_maps, list(range(NCORES)))
    out = np.stack([res.results[c]["outT"] for c in range(NCORES)])  # [8,4,D,N]
    return np.ascontiguousarray(
        out.transpose(0, 1, 3, 2).reshape(B, N, D)).astype(np.float32)
